# revision 2
# baseline (speedup 1.0000x reference)
"""Trainium2 Bass kernel for a single-layer batch-first GRU (PyTorch gate order).

Problem: noise (256, 2048, 10) -> GRU(10 -> 64) -> out (256, 2048, 64), f32.

Strategy: TIME-sharded across the 8 cores. The GRU forgets its state at a
rate of ~prod(z_t) (z ~= sigmoid(+-0.3) ~= 0.5), so core c computes the full
256-row batch for payload t in [c*256, (c+1)*256) after a 32-step warmup from
h=0 whose output is discarded; the truncation error decays ~0.5^32. Core 0's
warmup runs on zero-noise, so the host recomputes the first 64 payload steps
exactly in fp32 and splices them over the device result.

Per core, everything is bf16 and gate-major:
  - The state tile st[128, 32, 256] holds per step slot s: partitions 0-63 =
    h_{k-1}, 64-73 = x_k (DMA'd noise), 74 = ones (biases), 75-127 = zeros.
  - One 128x64-tiled matmul per gate computes the FULL pre-activation
    (hidden + input + bias) in one shot from the augmented stationary
    [W_h; W_i; b; 0]: z -> psum[0:64], r -> psum[64:128] of a zr slot.
  - n's hidden part (W_hn h) is a third matmul into ps_ng[0:64]; its input
    part gn is bulk-matmul'ed 4 steps at a time into ps_ng[64:128].
  - ACT sigmoid(zr) -> sbuf bf16; DVE m=(nh+b_hn)*r, s2=m+gn; ACT tanh(s2)
    -> psum (overwriting the dead nh slot); DVE q=(z-1)*n, p=z*h,
    h' = p - q -> next state slot. sigmoid and tanh share one ACT table set.
"""

import numpy as np
from contextlib import ExitStack

import ml_dtypes
import concourse.bass as bass
import concourse.tile as tile
from concourse import mybir
from concourse.bass_utils import run_bass_kernel_spmd

F32 = mybir.dt.float32
BF16 = mybir.dt.bfloat16
AF = mybir.ActivationFunctionType
OP = mybir.AluOpType

B, T, NI, NH = 256, 2048, 10, 64
NCORES = 8
WARM = 32                 # discarded warmup steps per segment
SEG = T // NCORES         # 256 payload steps per core
KTOT = SEG + WARM         # 288 total steps per core
SPLICE = 64               # host-recomputed exact prefix (core 0 fixup)

SR = 32                   # state ring slots (h + x staging)
ZR = 4                    # psum zr ring slots
NR = 8                    # psum ng ring slots (gn bulk in halves of 4)
XCH = 16                  # steps per noise DMA
OCH = 8                   # steps per output DMA flush

TRACE = False
_LAST_RESULTS = {}


def _split_excess_waits(nc, cap=1):
    """walrus (CoreV3) rejects instructions carrying more than `cap` sem
    waits; hoist the excess onto same-engine NoOps just before."""
    for f in nc.m.functions:
        for bb in f.blocks:
            new_insts = []
            for inst in bb.instructions:
                si = inst.sync_info
                if si and si.on_wait and len(si.on_wait) > cap:
                    waits = list(si.on_wait)
                    extra, keep = waits[:-cap], waits[-cap:]
                    for k, i in enumerate(range(0, len(extra), cap)):
                        nop = mybir.InstNoOp(
                            name=f"{inst.name}_ws{k}", ins=[], outs=[]
                        )
                        nop.engine = inst.engine
                        nop.sync_info = mybir.SyncInfo(
                            on_wait=extra[i : i + cap], on_update=[]
                        )
                        new_insts.append(nop)
                    si.on_wait = keep
                new_insts.append(inst)
            bb.instructions = new_insts
    return nc


def _build():
    nc = bass.Bass("TRN2", target_bir_lowering=False, debug=False)

    x_d = nc.declare_dram_parameter("xT", [NI + 1, KTOT, B], BF16, False)
    # stacked 128x64 stationaries: 0=z_aug, 1=r_aug, 2=nh, 3=gn_aug
    w_d = nc.declare_dram_parameter("wstk", [128, 4, NH], BF16, False)
    bhn_d = nc.declare_dram_parameter("bhn", [128, 1], F32, False)
    out_d = nc.declare_dram_parameter("outT", [NH, SEG, B], BF16, True)

    with tile.TileContext(nc) as tc, ExitStack() as ctx:
        const = ctx.enter_context(tc.tile_pool(name="const", bufs=1))
        work = ctx.enter_context(tc.tile_pool(name="work", bufs=3))
        psum = ctx.enter_context(tc.tile_pool(name="psum", bufs=1, space="PSUM"))

        wsb = const.tile([128, 4, NH], BF16)
        nc.sync.dma_start(out=wsb, in_=w_d[:])
        bhn = const.tile([128, 1], F32)
        nc.sync.dma_start(out=bhn, in_=bhn_d[:])

        # state ring: [0:64]=h, [64:74]=x, [74]=ones (via DMA), [75:128]=zeros
        # Engine memsets must start on 32-aligned partitions, so zero both
        # halves wholesale; the x DMA (rows 64..74) lands on top. The h
        # region must be finite everywhere: bulk matmuls stream stale h
        # slots against zero weight rows, and NaN * 0 = NaN.
        st = const.tile([128, SR, B], BF16)
        nc.vector.memset(st[0:64, :, :], 0.0)
        nc.vector.memset(st[64:128, :, :], 0.0)

        ps_zr = psum.tile([128, ZR, B], F32)   # 2 banks
        ps_ng = psum.tile([128, NR, B], F32)   # 4 banks

        def dma_x(k0, n):
            s = k0 % SR
            nc.sync.dma_start(
                out=st[64 : 64 + NI + 1, s : s + n, :],
                in_=x_d[:, k0 : k0 + n, :],
            )

        def bulk_gn(k0):
            # input projections for steps [k0, k0+4) -> ps_ng[64:128]
            s8 = k0 % NR
            s32 = k0 % SR
            for j in (0, 1):
                nc.tensor.matmul(
                    ps_ng[64:128, s8 + 2 * j : s8 + 2 * j + 2, :],
                    wsb[:, 3, :],
                    st[:, s32 + 2 * j : s32 + 2 * j + 2, :],
                    start=True, stop=True,
                    tile_position=(0, 64),
                    skip_group_check=True,
                )

        # prologue: stage x for steps [0, 32), gn for [0, 4)
        dma_x(0, XCH)
        dma_x(XCH, XCH)
        bulk_gn(0)

        for k in range(KTOT):
            s4, s8, s32 = k % ZR, k % NR, k % SR
            # stage the next chunk only once the slots it reuses are fully
            # consumed: at the first step of chunk c, slots of chunk c-2 are
            # free (their last reader is step k-1, already emitted)
            if k % XCH == 0 and k > 0 and k + 2 * XCH <= KTOT:
                dma_x(k + XCH, XCH)
            if k % 4 == 0 and k + 8 <= KTOT:
                bulk_gn(k + 4)  # steps [k+4, k+8)

            rhs = st[:, s32, :]
            nc.tensor.matmul(
                ps_zr[0:64, s4, :], wsb[:, 0, :], rhs,
                start=True, stop=True, tile_position=(0, 0),
                skip_group_check=True,
            )
            nc.tensor.matmul(
                ps_zr[64:128, s4, :], wsb[:, 1, :], rhs,
                start=True, stop=True, tile_position=(0, 64),
                skip_group_check=True,
            )
            nc.tensor.matmul(
                ps_ng[0:64, s8, :], wsb[:, 2, :], rhs,
                start=True, stop=True, tile_position=(0, 0),
                skip_group_check=True,
            )

            zr = work.tile([128, B], BF16, tag="zr")
            nc.scalar.activation(zr, ps_zr[:, s4, :], AF.Sigmoid)
            m = work.tile([128, B], BF16, tag="m")
            nc.vector.scalar_tensor_tensor(
                m[64:128, :], ps_ng[0:64, s8, :], bhn[64:128, :],
                zr[64:128, :], OP.add, OP.mult,
            )
            s2 = work.tile([128, B], BF16, tag="s2")
            nc.vector.tensor_tensor(
                s2[64:128, :], m[64:128, :], ps_ng[64:128, s8, :], OP.add
            )
            nc.scalar.activation(ps_ng[0:64, s8, :], s2[64:128, :], AF.Tanh)
            p = work.tile([64, B], BF16, tag="p")
            nc.vector.tensor_mul(p, zr[0:64, :], st[0:64, s32, :])
            q = work.tile([64, B], BF16, tag="q")
            nc.vector.scalar_tensor_tensor(
                q, zr[0:64, :], 1.0, ps_ng[0:64, s8, :],
                OP.subtract, OP.mult,
            )
            nc.vector.tensor_tensor(
                st[0:64, (k + 1) % SR, :], p, q, OP.subtract
            )

            if (k + 1) % OCH == 0 and k + 1 > WARM:
                # flush h_j for j in [a, a+8); h_j lives at slot (j+1) % SR,
                # so the slot window [a+1, a+9) can wrap the ring once.
                a = k + 1 - OCH
                o0 = a - WARM
                s0 = (a + 1) % SR
                n1 = min(OCH, SR - s0)
                nc.sync.dma_start(
                    out=out_d[:, o0 : o0 + n1, :],
                    in_=st[0:64, s0 : s0 + n1, :],
                )
                if n1 < OCH:
                    nc.sync.dma_start(
                        out=out_d[:, o0 + n1 : o0 + OCH, :],
                        in_=st[0:64, 0 : OCH - n1, :],
                    )

    _split_excess_waits(nc)
    return nc


_NC_CACHE = []


def _get_nc():
    if not _NC_CACHE:
        _NC_CACHE.append(_build())
    return _NC_CACHE[0]


def _bf16(x):
    return np.asarray(x, np.float32).astype(ml_dtypes.bfloat16)


def _gru_prefix(noise, w_ih, w_hh, b_ih, b_hh, nsteps):
    """Exact fp32 GRU for the first nsteps, all batch rows."""
    H = NH
    w_hr, w_hz, w_hn = w_hh[0:H], w_hh[H : 2 * H], w_hh[2 * H :]
    b_hr, b_hz, b_hn = b_hh[0:H], b_hh[H : 2 * H], b_hh[2 * H :]
    gi = np.einsum("bti,gi->btg", noise[:, :nsteps], w_ih) + b_ih
    h = np.zeros((noise.shape[0], H), np.float32)
    out = np.empty((noise.shape[0], nsteps, H), np.float32)
    for t in range(nsteps):
        g = gi[:, t]
        g_r, g_z, g_n = g[:, 0:H], g[:, H : 2 * H], g[:, 2 * H :]
        r = 1.0 / (1.0 + np.exp(-(g_r + h @ w_hr.T + b_hr)))
        z = 1.0 / (1.0 + np.exp(-(g_z + h @ w_hz.T + b_hz)))
        n = np.tanh(g_n + r * (h @ w_hn.T + b_hn))
        h = z * h + (1.0 - z) * n
        out[:, t] = h
    return out


def kernel(noise, w_ih, w_hh, b_ih, b_hh):
    noise = np.ascontiguousarray(np.asarray(noise, dtype=np.float32))
    w_ih = np.asarray(w_ih, dtype=np.float32)
    w_hh = np.asarray(w_hh, dtype=np.float32)
    b_ih = np.asarray(b_ih, dtype=np.float32)
    b_hh = np.asarray(b_hh, dtype=np.float32)

    H = NH
    w_ihT, w_hhT = w_ih.T, w_hh.T  # (10, 192), (64, 192)
    # PyTorch gate order: [0:H]=r, [H:2H]=z, [2H:3H]=n
    blocks = np.zeros((128, 4, H), np.float32)
    blocks[0:64, 0, :] = w_hhT[:, H : 2 * H]          # z hidden
    blocks[64:74, 0, :] = w_ihT[:, H : 2 * H]
    blocks[74, 0, :] = b_ih[H : 2 * H] + b_hh[H : 2 * H]
    blocks[0:64, 1, :] = w_hhT[:, 0:H]                # r hidden
    blocks[64:74, 1, :] = w_ihT[:, 0:H]
    blocks[74, 1, :] = b_ih[0:H] + b_hh[0:H]
    blocks[0:64, 2, :] = w_hhT[:, 2 * H :]            # n hidden
    blocks[64:74, 3, :] = w_ihT[:, 2 * H :]           # n input
    blocks[74, 3, :] = b_ih[2 * H :]
    bhn = np.zeros((128, 1), np.float32)
    bhn[64:128, 0] = b_hh[2 * H :]

    noiseT = noise.transpose(2, 1, 0)  # (10, T, B)
    padded = np.concatenate(
        [np.zeros((NI, WARM, B), np.float32), noiseT], axis=1
    )  # (10, WARM+T, B)
    padded = np.concatenate(
        [padded, np.ones((1, WARM + T, B), np.float32)], axis=0
    )  # row NI = ones (feeds the bias row of the stationaries)

    wstk = _bf16(blocks)
    shared = {"wstk": wstk, "bhn": bhn}
    in_maps = []
    for c in range(NCORES):
        x_c = _bf16(padded[:, c * SEG : c * SEG + KTOT, :])
        in_maps.append({"xT": np.ascontiguousarray(x_c), **shared})

    nc = _get_nc()
    res = run_bass_kernel_spmd(
        nc, in_maps, core_ids=list(range(NCORES)), trace=TRACE
    )
    _LAST_RESULTS["res"] = res

    out = np.empty((B, T, H), dtype=np.float32)
    for c in range(NCORES):
        seg = np.asarray(res.results[c]["outT"]).astype(np.float32)
        out[:, c * SEG : (c + 1) * SEG, :] = seg.transpose(2, 1, 0)
    # core 0's warmup ran on zero-noise; splice the exact prefix
    out[:, :SPLICE, :] = _gru_prefix(noise, w_ih, w_hh, b_ih, b_hh, SPLICE)
    return out


# revision 3
# speedup vs baseline: 1.0004x; 1.0004x over previous
"""Trainium2 Bass kernel for a single-layer batch-first GRU (PyTorch gate order).

Problem: noise (256, 2048, 10) -> GRU(10 -> 64) -> out (256, 2048, 64), f32.

v3: TIME-sharded 16 ways; each of the 8 cores runs TWO independent
time-segment chains (A/B) interleaved, hiding the serial-chain latency of one
behind the other. The GRU forgets its state at ~prod(z_t) (z ~= 0.5/step), so
each segment starts from h=0 with a 32-step discarded warmup; truncation error
~0.5^32. Segment 0 warms up on zero-noise and the host splices the first 64
payload steps exactly in fp32.

Per chain, everything is bf16 and gate-major:
  - state ring st[128, 32, 256]: partitions 0-63 = h_{k-1}, 64-73 = x_k
    (DMA'd noise), 74 = ones, 75-127 = zeros.
  - One 128x64-tiled matmul per gate gives the FULL pre-activation (hidden +
    input + bias) from the augmented stationary [W_h; W_i; b; 0]:
    z -> psum[0:64], r -> psum[64:128]. n's hidden part is a third matmul;
    its input part gn is bulk-matmul'ed 2 steps ahead and staged to SBUF
    bf16 by an ACT copy so the s2 add runs in DVE 2x mode.
  - chain: sigmoid(zr) -> m=(nh+b_hn)*r -> s2=m+gn -> tanh -> psum ->
    q=(z-1)*n, h'=p-q; p=z*h runs on GpSimd during the tanh window.
"""

import numpy as np
from contextlib import ExitStack

import ml_dtypes
import concourse.bass as bass
import concourse.tile as tile
from concourse import mybir
from concourse.bass_utils import run_bass_kernel_spmd

F32 = mybir.dt.float32
BF16 = mybir.dt.bfloat16
AF = mybir.ActivationFunctionType
OP = mybir.AluOpType

B, T, NI, NH = 256, 2048, 10, 64
NCORES = 8
NSEG = 16                 # time segments (2 per core)
WARM = 16                 # discarded warmup steps per segment
SEG = T // NSEG           # 128 payload steps per segment
KTOT = SEG + WARM         # 160 steps per chain
SPLICE = 64               # host-recomputed exact prefix (segment 0 fixup)

SR = 32                   # state ring slots
ZR = 2                    # psum zr ring slots (1 bank)
NR = 4                    # psum ng ring slots (2 banks), bulk in halves of 2
XCH = 16                  # steps per noise DMA
OCH = 8                   # steps per output DMA flush

TRACE = False
_LAST_RESULTS = {}


def _split_excess_waits(nc, cap=1):
    """walrus (CoreV3) rejects instructions carrying more than `cap` sem
    waits; hoist the excess onto same-engine NoOps just before."""
    for f in nc.m.functions:
        for bb in f.blocks:
            new_insts = []
            for inst in bb.instructions:
                si = inst.sync_info
                if si and si.on_wait and len(si.on_wait) > cap:
                    waits = list(si.on_wait)
                    extra, keep = waits[:-cap], waits[-cap:]
                    for k, i in enumerate(range(0, len(extra), cap)):
                        nop = mybir.InstNoOp(
                            name=f"{inst.name}_ws{k}", ins=[], outs=[]
                        )
                        nop.engine = inst.engine
                        nop.sync_info = mybir.SyncInfo(
                            on_wait=extra[i : i + cap], on_update=[]
                        )
                        new_insts.append(nop)
                    si.on_wait = keep
                new_insts.append(inst)
            bb.instructions = new_insts
    return nc


def _build():
    nc = bass.Bass("TRN2", target_bir_lowering=False, debug=False)

    x_d = nc.declare_dram_parameter("xT", [2, NI + 1, KTOT, B], BF16, False)
    # stacked 128x64 stationaries: 0=z_aug, 1=r_aug, 2=nh, 3=gn_aug
    w_d = nc.declare_dram_parameter("wstk", [128, 4, NH], BF16, False)
    out_d = nc.declare_dram_parameter("outT", [NH, 2, SEG, B], BF16, True)

    with tile.TileContext(nc) as tc, ExitStack() as ctx:
        const = ctx.enter_context(tc.tile_pool(name="const", bufs=1))
        work = ctx.enter_context(tc.tile_pool(name="work", bufs=4))
        psum = ctx.enter_context(tc.tile_pool(name="psum", bufs=1, space="PSUM"))

        wsb = const.tile([128, 4, NH], BF16)
        nc.sync.dma_start(out=wsb, in_=w_d[:])

        chains = []
        for j, tag in ((0, "A"), (1, "B")):
            st = const.tile([128, SR, B], BF16, name=f"st{tag}")
            nc.vector.memset(st[0:64, :, :], 0.0)
            nc.vector.memset(st[64:128, :, :], 0.0)
            gn_sb = const.tile([128, NR, B], BF16, name=f"gn{tag}")
            ps_zr = psum.tile([128, ZR, B], F32, name=f"zr{tag}")
            ps_ng = psum.tile([128, NR, B], F32, name=f"ng{tag}")
            chains.append(dict(j=j, tag=tag, st=st, gn_sb=gn_sb,
                               ps_zr=ps_zr, ps_ng=ps_ng))

        def dma_x(c, k0, n):
            s = k0 % SR
            nc.sync.dma_start(
                out=c["st"][64 : 64 + NI + 1, s : s + n, :],
                in_=x_d[c["j"], :, k0 : k0 + n, :],
            )

        def bulk_gn(c, k0):
            # input projections for steps [k0, k0+2) -> ps_ng[64:128]
            s4, s32 = k0 % NR, k0 % SR
            nc.tensor.matmul(
                c["ps_ng"][64:128, s4 : s4 + 2, :],
                wsb[:, 3, :],
                c["st"][:, s32 : s32 + 2, :],
                start=True, stop=True,
                tile_position=(0, 64),
                skip_group_check=True,
            )

        def copy_gn(c, k0):
            s4 = k0 % NR
            nc.scalar.copy(
                c["gn_sb"][64:128, s4 : s4 + 2, :],
                c["ps_ng"][64:128, s4 : s4 + 2, :],
            )

        for c in chains:
            dma_x(c, 0, XCH)
            dma_x(c, XCH, XCH)
            bulk_gn(c, 0)
            copy_gn(c, 0)


        def front(c, k):
            s2r, s4, s32 = k % ZR, k % NR, k % SR
            st, gn_sb = c["st"], c["gn_sb"]
            ps_zr, ps_ng = c["ps_zr"], c["ps_ng"]
            tag = c["tag"]
            if k % XCH == 0 and k > 0 and k + 2 * XCH <= KTOT:
                dma_x(c, k + XCH, XCH)
            do_bulk = k % 2 == 0 and k + 4 <= KTOT
            if do_bulk:
                bulk_gn(c, k + 2)
            rhs = st[:, s32, :]
            nc.tensor.matmul(
                ps_zr[0:64, s2r, :], wsb[:, 0, :], rhs,
                start=True, stop=True, tile_position=(0, 0),
                skip_group_check=True,
            )
            nc.tensor.matmul(
                ps_zr[64:128, s2r, :], wsb[:, 1, :], rhs,
                start=True, stop=True, tile_position=(0, 64),
                skip_group_check=True,
            )
            nc.tensor.matmul(
                ps_ng[0:64, s4, :], wsb[:, 2, :], rhs,
                start=True, stop=True, tile_position=(0, 0),
                skip_group_check=True,
            )
            zr = work.tile([128, B], BF16, tag=f"zr{tag}")
            nc.scalar.activation(zr, ps_zr[:, s2r, :], AF.Sigmoid)
            if do_bulk:
                copy_gn(c, k + 2)
            m = work.tile([128, B], BF16, tag=f"m{tag}")
            nc.vector.tensor_tensor(
                m[64:128, :], ps_ng[0:64, s4, :], zr[64:128, :], OP.mult
            )
            s2 = work.tile([128, B], BF16, tag=f"s2{tag}")
            nc.vector.tensor_tensor(
                s2[64:128, :], m[64:128, :], gn_sb[64:128, s4, :], OP.add
            )
            c["zr"], c["s2"] = zr, s2

        def back(c, k):
            s4, s32 = k % NR, k % SR
            st = c["st"]
            ps_ng = c["ps_ng"]
            tag = c["tag"]
            zr, s2 = c["zr"], c["s2"]
            nc.scalar.activation(ps_ng[0:64, s4, :], s2[64:128, :], AF.Tanh)
            p = work.tile([64, B], BF16, tag=f"p{tag}")
            nc.gpsimd.tensor_mul(p, zr[0:64, :], st[0:64, s32, :])
            q = work.tile([64, B], BF16, tag=f"q{tag}")
            nc.vector.scalar_tensor_tensor(
                q, zr[0:64, :], 1.0, ps_ng[0:64, s4, :],
                OP.subtract, OP.mult,
            )
            nc.vector.tensor_tensor(
                st[0:64, (k + 1) % SR, :], p, q, OP.subtract
            )
            if (k + 1) % OCH == 0 and k + 1 > WARM:
                a = k + 1 - OCH
                o0 = a - WARM
                s0 = (a + 1) % SR
                n1 = min(OCH, SR - s0)
                nc.sync.dma_start(
                    out=out_d[:, c["j"], o0 : o0 + n1, :],
                    in_=st[0:64, s0 : s0 + n1, :],
                )
                if n1 < OCH:
                    nc.sync.dma_start(
                        out=out_d[:, c["j"], o0 + n1 : o0 + OCH, :],
                        in_=st[0:64, 0 : OCH - n1, :],
                    )

        # anti-phased emission: every engine FIFO alternates between the two
        # chains at half-step granularity, forcing them ~half a period apart
        cA, cB = chains
        for k in range(KTOT):
            front(cA, k)
            if k > 0:
                back(cB, k - 1)
            back(cA, k)
            front(cB, k)
        back(cB, KTOT - 1)

    _split_excess_waits(nc)
    return nc


_NC_CACHE = []


def _get_nc():
    if not _NC_CACHE:
        _NC_CACHE.append(_build())
    return _NC_CACHE[0]


def _bf16(x):
    return np.asarray(x, np.float32).astype(ml_dtypes.bfloat16)


def _gru_prefix(noise, w_ih, w_hh, b_ih, b_hh, nsteps):
    """Exact fp32 GRU for the first nsteps, all batch rows."""
    H = NH
    w_hr, w_hz, w_hn = w_hh[0:H], w_hh[H : 2 * H], w_hh[2 * H :]
    b_hr, b_hz, b_hn = b_hh[0:H], b_hh[H : 2 * H], b_hh[2 * H :]
    gi = np.einsum("bti,gi->btg", noise[:, :nsteps], w_ih) + b_ih
    h = np.zeros((noise.shape[0], H), np.float32)
    out = np.empty((noise.shape[0], nsteps, H), np.float32)
    for t in range(nsteps):
        g = gi[:, t]
        g_r, g_z, g_n = g[:, 0:H], g[:, H : 2 * H], g[:, 2 * H :]
        r = 1.0 / (1.0 + np.exp(-(g_r + h @ w_hr.T + b_hr)))
        z = 1.0 / (1.0 + np.exp(-(g_z + h @ w_hz.T + b_hz)))
        n = np.tanh(g_n + r * (h @ w_hn.T + b_hn))
        h = z * h + (1.0 - z) * n
        out[:, t] = h
    return out


def kernel(noise, w_ih, w_hh, b_ih, b_hh):
    noise = np.ascontiguousarray(np.asarray(noise, dtype=np.float32))
    w_ih = np.asarray(w_ih, dtype=np.float32)
    w_hh = np.asarray(w_hh, dtype=np.float32)
    b_ih = np.asarray(b_ih, dtype=np.float32)
    b_hh = np.asarray(b_hh, dtype=np.float32)

    H = NH
    w_ihT, w_hhT = w_ih.T, w_hh.T
    # PyTorch gate order: [0:H]=r, [H:2H]=z, [2H:3H]=n
    blocks = np.zeros((128, 4, H), np.float32)
    blocks[0:64, 0, :] = w_hhT[:, H : 2 * H]          # z hidden
    blocks[64:74, 0, :] = w_ihT[:, H : 2 * H]
    blocks[74, 0, :] = b_ih[H : 2 * H] + b_hh[H : 2 * H]
    blocks[0:64, 1, :] = w_hhT[:, 0:H]                # r hidden
    blocks[64:74, 1, :] = w_ihT[:, 0:H]
    blocks[74, 1, :] = b_ih[0:H] + b_hh[0:H]
    blocks[0:64, 2, :] = w_hhT[:, 2 * H :]            # n hidden
    blocks[74, 2, :] = b_hh[2 * H :]                  # b_hn rides the ones-row
    blocks[64:74, 3, :] = w_ihT[:, 2 * H :]           # n input
    blocks[74, 3, :] = b_ih[2 * H :]
    noiseT = noise.transpose(2, 1, 0)  # (10, T, B)
    padded = np.concatenate(
        [np.zeros((NI, WARM, B), np.float32), noiseT], axis=1
    )
    padded = np.concatenate(
        [padded, np.ones((1, WARM + T, B), np.float32)], axis=0
    )  # (11, WARM+T, B); row NI = ones (feeds the bias stationary row)

    wstk = _bf16(blocks)
    shared = {"wstk": wstk}
    in_maps = []
    for c in range(NCORES):
        xs = []
        for j in range(2):
            seg = 2 * c + j
            xs.append(padded[:, seg * SEG : seg * SEG + KTOT, :])
        x_c = _bf16(np.stack(xs, axis=0))  # (2, 11, KTOT, B)
        in_maps.append({"xT": np.ascontiguousarray(x_c), **shared})

    nc = _get_nc()
    res = run_bass_kernel_spmd(
        nc, in_maps, core_ids=list(range(NCORES)), trace=TRACE
    )
    _LAST_RESULTS["res"] = res

    out = np.empty((B, T, H), dtype=np.float32)
    for c in range(NCORES):
        seg_out = np.asarray(res.results[c]["outT"]).astype(np.float32)
        for j in range(2):
            seg = 2 * c + j
            out[:, seg * SEG : (seg + 1) * SEG, :] = (
                seg_out[:, j].transpose(2, 1, 0)
            )
    # segment 0's warmup ran on zero-noise; splice the exact prefix
    out[:, :SPLICE, :] = _gru_prefix(noise, w_ih, w_hh, b_ih, b_hh, SPLICE)
    return out
